# revision 20
# baseline (speedup 1.0000x reference)
"""ChebGNN encoder (3x ChebConv(K=5) + Linear skip + ReLU) on 8 Trainium2
NeuronCores.

Strategy
--------
* Nodes are sharded over the 8 cores (graph parallel). Each core owns
  TPC dest-tiles of 128 node slots (padded rows; a host-chosen permutation
  bin-packs nodes into tiles so every (core, tile) has the same number of
  128-edge chunks).
* Each Chebyshev layer is evaluated with the Clenshaw recursion
      b4 = Z4;  b_k = Z_k + 2 L b_{k+1} - b_{k+2};  out = Z0 + L b1 - b2
  where Z_k = h @ Wc[k] (Z0 additionally fuses the Linear skip + biases,
  via host-side weight fusion and a K=1 ones x bias matmul).
* The sparse propagation L @ b is computed per dest-tile as
      psum += S_j.T @ X_j
  with S_j a [128 edge, 128 dest] bf16 block holding 2*w_edge
  (host-precomputed from edge_index) and X_j = dma_gather of the 128
  source rows from the all-gathered state in HBM.
* The state exchange is a split AllGather: the shard's tiles 0-9 (half A)
  are exchanged as soon as they are produced, so the half-A transfer
  overlaps the tail of the producing stage and the half-B transfer overlaps
  the next stage's half-A sparse work. Edge chunks are grouped by source
  half on the host, so half-A sparse matmuls only need the half-A exchange.
* Layers 0/1 exchange the b-state in fp8-e3m4; the state is kept scaled
  (s0=1/2, s1=1/4, folded into the dense weights host-side) so it fits
  e3m4's +-15.5 range. The sparse matmul takes the fp8 gather output
  directly (bf16 S x fp8 X). Layer 2 exchanges 64-wide bf16 and widens to
  256B rows for the gather.
"""

import numpy as np
import ml_dtypes

BF16 = ml_dtypes.bfloat16

# ---------------------------------------------------------------- config ---

N = 20000
E = 320000
F_IN = 350
HID = 256
BOT = 64
K = 5
NCORES = 8
CHUNK = 128
TPC = 20                    # dest tiles per core
GTI = 4                     # dest tiles per gather group
HALF = TPC // 2             # tiles per exchange half
HROWS = HALF * CHUNK        # rows per half (1280)
NROWS = TPC * CHUNK         # padded rows per core (2560)
KPAD_IN = 384               # F_IN padded to 3*128
F2PAD = 128                 # layer-2 gather row width (256B rows)
F2REAL = 64                 # layer-2 compute width (BOT=64, no padding)

# Per-layer scale of the b-state space: layers 0/1 exchange b in fp8-e3m4
# (max 15.5), so b is kept scaled to fit: |b0|<12 -> s0=1/2, |b1|<51 -> s1=1/4.
# The scale is folded into the dense weights (W_li *= s_li/s_{li-1}), so the
# device-side recursion/prop is identical to the unscaled bf16 version.
SCALES = (0.5, 0.25, 1.0)


def _layer_dims(li):
    # (kpad = padded contraction dim, f_out = compute output width)
    return ((KPAD_IN, HID), (HID, HID), (HID, F2REAL))[li]


# ------------------------------------------------------ host preprocessing ---


def _edge_norm_host(edge_index):
    row = edge_index[0].astype(np.int64)
    col = edge_index[1].astype(np.int64)
    mask = row != col
    deg = np.bincount(row[mask], minlength=N).astype(np.float32)
    with np.errstate(divide="ignore"):
        dinv = np.where(deg > 0, 1.0 / np.sqrt(np.maximum(deg, 1e-12)), 0.0).astype(
            np.float32
        )
    w = (-dinv[row] * dinv[col]).astype(np.float32)
    w = np.where(mask, w, 0.0)
    return row, col, w, mask


def _build_all(edge_index):
    """Node permutation + per-core padded edge structure, split by source half.

    Returns (meta, cores): meta has gslot [N], slot_node, cpa, cpb; cores[c]
    has ((idxA, svalA, dlocA), (idxB, svalB, dlocB)).
    """
    row, col, w, mask = _edge_norm_host(edge_index)
    indeg = np.bincount(col[mask], minlength=N).astype(np.int64)

    nloc = N // NCORES
    order = np.argsort(-indeg, kind="stable")
    node_core = np.empty(N, dtype=np.int64)
    rounds = np.arange(N) // NCORES
    fwd = np.arange(N) % NCORES
    snake = np.where(rounds % 2 == 0, fwd, NCORES - 1 - fwd)
    node_core[order] = snake

    tile_of_node = np.empty(N, dtype=np.int64)
    slot_of_node = np.empty(N, dtype=np.int64)
    for c in range(NCORES):
        nodes_c = np.flatnonzero(node_core == c)
        assert len(nodes_c) == nloc
        heavy_first = nodes_c[np.argsort(-indeg[nodes_c], kind="stable")]
        loads = np.zeros(TPC, dtype=np.int64)
        counts = np.zeros(TPC, dtype=np.int64)
        for nd in heavy_first:
            cand = np.flatnonzero(counts < CHUNK)
            t = cand[np.argmin(loads[cand])]
            tile_of_node[nd] = t
            slot_of_node[nd] = counts[t]
            counts[t] += 1
            loads[t] += indeg[nd]

    gslot = node_core * NROWS + tile_of_node * CHUNK + slot_of_node
    slot_node = np.full(NCORES * NROWS, -1, dtype=np.int64)
    slot_node[gslot] = np.arange(N)

    er, ec, ew = row[mask], col[mask], w[mask]
    dest_core = node_core[ec]
    dest_tile = tile_of_node[ec]
    dest_slot = slot_of_node[ec]
    src_gslot = gslot[er]
    src_core = src_gslot // NROWS
    src_loc = src_gslot % NROWS
    src_half = src_loc >= HROWS  # False=A (tiles 0-9), True=B
    # row index within the half-exchange buffer [NCORES*HROWS, fo]
    src_hrow = src_core * HROWS + np.where(src_half, src_loc - HROWS, src_loc)

    # global chunk counts per half
    cnt = np.zeros((2, NCORES, TPC), dtype=np.int64)
    np.add.at(cnt, (src_half.astype(np.int64), dest_core, dest_tile), 1)
    cpa = int(np.ceil(cnt[0].max() / CHUNK))
    cpb = int(np.ceil(cnt[1].max() / CHUNK))

    cores = []
    for c in range(NCORES):
        m = dest_core == c
        halves = []
        for h, cp in ((0, cpa), (1, cpb)):
            mh = m & (src_half == (h == 1))
            t_arr = dest_tile[mh]
            s_arr = dest_slot[mh]
            src_arr = src_hrow[mh]
            w_arr = ew[mh]
            o = np.argsort(t_arr, kind="stable")
            t_arr, s_arr, src_arr, w_arr = t_arr[o], s_arr[o], src_arr[o], w_arr[o]
            epad = TPC * cp * CHUNK
            idx = np.zeros(epad, dtype=np.int16)
            sval = np.zeros(epad, dtype=np.float32)
            dloc = np.zeros(epad, dtype=np.int16)
            starts = np.searchsorted(t_arr, np.arange(TPC))
            ends = np.searchsorted(t_arr, np.arange(TPC) + 1)
            for t in range(TPC):
                a, b = starts[t], ends[t]
                base = t * cp * CHUNK
                idx[base : base + b - a] = src_arr[a:b]
                sval[base : base + b - a] = 2.0 * w_arr[a:b]
                dloc[base : base + b - a] = s_arr[a:b]
            halves.append((idx, sval, dloc))
        cores.append(halves)

    meta = dict(gslot=gslot, slot_node=slot_node, cpa=cpa, cpb=cpb)
    return meta, cores


def _pack_idx(idx, piece):
    """dma_gather index layout: flat gathered position i within a piece reads
    idx_sb[i % 16, i // 16] (column-major wrap over 16 partitions), pieces
    side by side along the free dim, replicated to 128 partitions."""
    epad = idx.shape[0]
    blocks = [
        np.ascontiguousarray(idx[g * piece : (g + 1) * piece].reshape(piece // 16, 16).T)
        for g in range(epad // piece)
    ]
    arr = np.concatenate(blocks, axis=1)
    return np.ascontiguousarray(np.tile(arr, (8, 1))).astype(np.int16)


def _build_sblocks(sval, dloc, epad):
    s = np.zeros((CHUNK, epad), dtype=np.float32)
    flat = np.arange(epad)
    j = flat // CHUNK
    kk = flat % CHUNK
    s[kk, j * CHUNK + dloc.astype(np.int64)] = sval
    return s.astype(BF16)


def _fuse_weights(inputs):
    """Per layer: (wf [kpad, f_out] = Wc[0]+Wl (+0 pad), wrest [4, kpad, f_out],
    bias [1, f_out] = bc+bl), all bf16, scaled by s_li/s_{li-1}."""
    out = []
    for li in range(3):
        kpad, f_out = _layer_dims(li)
        gain = SCALES[li] / (SCALES[li - 1] if li > 0 else 1.0)
        Wc = np.asarray(inputs[f"Wc{li}"], np.float32) * gain
        Wl = np.asarray(inputs[f"Wl{li}"], np.float32) * gain
        bc = np.asarray(inputs[f"bc{li}"], np.float32) * SCALES[li]
        bl = np.asarray(inputs[f"bl{li}"], np.float32) * SCALES[li]
        f_in, f_real = Wc.shape[1], Wc.shape[2]
        wf = np.zeros((kpad, f_out), np.float32)
        wf[:f_in, :f_real] = Wc[0] + Wl
        wrest = np.zeros((K - 1, kpad, f_out), np.float32)
        for k in range(1, K):
            wrest[k - 1, :f_in, :f_real] = Wc[k]
        bias = np.zeros((1, f_out), np.float32)
        bias[0, :f_real] = bc + bl
        out.append((wf.astype(BF16), wrest.astype(BF16), bias.astype(BF16)))
    return out


# ------------------------------------------------------------- device side ---


NQUEUES = 4


def build_bass(cpa, cpb, reps=1, ablate=()):
    import concourse.bacc as bacc
    import concourse.bass as bass
    import concourse.mybir as mybir
    import concourse.tile as tile
    from concourse import library_config

    dt = mybir.dt
    epad_a = TPC * cpa * CHUNK
    epad_b = TPC * cpb * CHUNK

    nc = bacc.Bacc(
        "TRN2",
        target_bir_lowering=False,
        debug=False,
        num_devices=NCORES,
        num_swdge_queues=NQUEUES,
    )

    # ---- I/O ----
    xT_d = nc.dram_tensor("xT", [KPAD_IN, NROWS], dt.bfloat16, kind="ExternalInput")
    idxA_d = nc.dram_tensor("idxA", [128, epad_a // 16], dt.int16, kind="ExternalInput")
    idxB_d = nc.dram_tensor("idxB", [128, epad_b // 16], dt.int16, kind="ExternalInput")
    sblkA_d = nc.dram_tensor("sblkA", [128, epad_a], dt.bfloat16, kind="ExternalInput")
    sblkB_d = nc.dram_tensor("sblkB", [128, epad_b], dt.bfloat16, kind="ExternalInput")
    ident_d = nc.dram_tensor("ident", [128, 128], dt.bfloat16, kind="ExternalInput")
    w_d = []
    for li in range(3):
        kpad, f_out = _layer_dims(li)
        w_d.append(
            (
                nc.dram_tensor(f"wf{li}", [kpad, f_out], dt.bfloat16, kind="ExternalInput"),
                nc.dram_tensor(
                    f"wr{li}", [K - 1, kpad, f_out], dt.bfloat16, kind="ExternalInput"
                ),
                nc.dram_tensor(f"bias{li}", [1, f_out], dt.bfloat16, kind="ExternalInput"),
            )
        )
    out_d = nc.dram_tensor("out_sh", [NROWS, F2REAL], dt.float32, kind="ExternalOutput")

    # internal DRAM for the split state exchange
    bnA8 = nc.dram_tensor("bnA8", [HROWS, HID], dt.float8e3, kind="Internal")
    bnB8 = nc.dram_tensor("bnB8", [HROWS, HID], dt.float8e3, kind="Internal")
    bfA8 = nc.dram_tensor(
        "bfA8", [NCORES * HROWS, HID], dt.float8e3, kind="Internal", addr_space="Shared"
    )
    bfB8 = nc.dram_tensor(
        "bfB8", [NCORES * HROWS, HID], dt.float8e3, kind="Internal", addr_space="Shared"
    )
    bnA2 = nc.dram_tensor("bnA2", [HROWS, F2REAL], dt.bfloat16, kind="Internal")
    bnB2 = nc.dram_tensor("bnB2", [HROWS, F2REAL], dt.bfloat16, kind="Internal")
    bfA264 = nc.dram_tensor(
        "bfA264", [NCORES * HROWS, F2REAL], dt.bfloat16, kind="Internal",
        addr_space="Shared",
    )
    bfB264 = nc.dram_tensor(
        "bfB264", [NCORES * HROWS, F2REAL], dt.bfloat16, kind="Internal",
        addr_space="Shared",
    )
    bfA2 = nc.dram_tensor("bfA2", [NCORES * HROWS, F2PAD], dt.bfloat16, kind="Internal")
    bfB2 = nc.dram_tensor("bfB2", [NCORES * HROWS, F2PAD], dt.bfloat16, kind="Internal")

    groups = [list(range(NCORES))]

    with tile.TileContext(nc) as tc:
        with (
            tc.tile_pool(name="const", bufs=1) as cpool,
            tc.tile_pool(name="hT", bufs=2) as hpool,
            tc.tile_pool(name="bstate", bufs=3) as bpool,
            tc.tile_pool(name="rpa", bufs=1) as rapool,
            tc.tile_pool(name="xbuf", bufs=2) as xpool,
            tc.tile_pool(name="x8buf", bufs=2) as x8pool,
            tc.tile_pool(name="small", bufs=1) as spool,
            tc.tile_pool(name="zpsum", bufs=2, space="PSUM") as zpool,
            tc.tile_pool(name="rpsum", bufs=3, space="PSUM") as rpool,
            tc.tile_pool(name="tpsum", bufs=2, space="PSUM") as tpool,
        ):
            nc.gpsimd.load_library(library_config.mlp)
            dummy_x = {}

            # ---- resident loads (hT0 + weights first: they gate layer-0 Z4) ----
            hT0 = cpool.tile([128, KPAD_IN // 128, NROWS], dt.bfloat16, tag="hT0")
            nc.sync.dma_start(hT0[:], xT_d.ap().rearrange("(c p) n -> p c n", p=128))

            w_sb = []
            for li in range(3):
                kpad, f_out = _layer_dims(li)
                kc = kpad // 128
                wf_sb = cpool.tile([128, kc, f_out], dt.bfloat16, tag=f"wf{li}")
                nc.sync.dma_start(
                    wf_sb[:], w_d[li][0].ap().rearrange("(c p) f -> p c f", p=128)
                )
                wr_sb = cpool.tile([128, K - 1, kc, f_out], dt.bfloat16, tag=f"wr{li}")
                nc.sync.dma_start(
                    wr_sb[:], w_d[li][1].ap().rearrange("k (c p) f -> p k c f", p=128)
                )
                bias_sb = cpool.tile([1, f_out], dt.bfloat16, tag=f"bias{li}")
                nc.sync.dma_start(bias_sb[:], w_d[li][2].ap())
                w_sb.append((wf_sb, wr_sb, bias_sb))

            sblkA_sb = cpool.tile([128, epad_a], dt.bfloat16, tag="sblkA")
            nc.sync.dma_start(sblkA_sb[:], sblkA_d.ap())
            sblkB_sb = cpool.tile([128, epad_b], dt.bfloat16, tag="sblkB")
            nc.sync.dma_start(sblkB_sb[:], sblkB_d.ap())
            idxA_sb = cpool.tile([128, epad_a // 16], dt.int16, tag="idxA")
            nc.sync.dma_start(idxA_sb[:], idxA_d.ap())
            idxB_sb = cpool.tile([128, epad_b // 16], dt.int16, tag="idxB")
            nc.sync.dma_start(idxB_sb[:], idxB_d.ap())
            ident_sb = cpool.tile([128, 128], dt.bfloat16, tag="ident")
            nc.sync.dma_start(ident_sb[:], ident_d.ap())
            ones_sb = cpool.tile([1, 128], dt.bfloat16, tag="ones")
            nc.vector.memset(ones_sb[:], 1.0)

            def dense_tile(zp, hT_in, li, widx, t, with_bias, stop=True):
                """psum[128 nodes, f_out] = h_tile @ W  (+ ones x bias)."""
                kpad, f_out = _layer_dims(li)
                kc = kpad // 128
                wf_sb, wr_sb, bias_sb = w_sb[li]
                zv = zp[:, :f_out]
                for c in range(kc):
                    lhsT = hT_in[:, c, t * 128 : (t + 1) * 128]
                    rhs = wf_sb[:, c, :] if widx == 0 else wr_sb[:, widx - 1, c, :]
                    nc.tensor.matmul(
                        zv,
                        lhsT,
                        rhs,
                        start=(c == 0),
                        stop=(stop and c == kc - 1 and not with_bias),
                    )
                if with_bias:
                    nc.tensor.matmul(
                        zv, ones_sb[:1, :], bias_sb[:1, :], start=False, stop=stop
                    )

            def ag_half(bounce, bfull, widen_to=None):
                if "coll" in ablate:
                    return
                with nc.named_scope("AG"):
                    nc.gpsimd.collective_compute(
                        "AllGather",
                        mybir.AluOpType.bypass,
                        replica_groups=groups,
                        ins=[bounce.ap().opt()],
                        outs=[bfull.ap().opt()],
                    )
                    if widen_to is not None:
                        nc.sync.dma_start(widen_to.ap()[:, :F2REAL], bfull.ap())

            def run_layer(li, hT_in, hT_out):
                kpad, f_out = _layer_dims(li)
                fo = f_out
                fp8 = li < 2
                fg = F2PAD if li == 2 else fo  # gather row width (256B granule)
                if fp8:
                    bnA, bnB, gfA, gfB = bnA8, bnB8, bfA8, bfB8
                    wdA = wdB = None
                    gsrcA, gsrcB = bfA8, bfB8
                else:
                    bnA, bnB, gfA, gfB = bnA2, bnB2, bfA264, bfB264
                    wdA, wdB = bfA2, bfB2
                    gsrcA, gsrcB = bfA2, bfB2

                def bounce_tile(t, src_tile):
                    bn = bnA if t < HALF else bnB
                    th = t % HALF
                    if fp8:
                        st = spool.tile([128, HID], dt.float8e3, tag="st8")
                        nc.vector.tensor_copy(st[:, :fo], src_tile)
                        nc.sync.dma_start(
                            bn.ap()[th * 128 : (th + 1) * 128, :fo], st[:, :fo]
                        )
                    else:
                        nc.sync.dma_start(
                            bn.ap()[th * 128 : (th + 1) * 128, :], src_tile
                        )

                def gather_group(g, which):
                    """One dma_gather covering GTI tiles [g*GTI, (g+1)*GTI).

                    X is always an fp8-typed [128, GTI*cp, 256] byte buffer
                    (256B per gathered row); layer 2's bf16 rows are bitcast
                    back at the matmul site."""
                    if which == 0:
                        cp, idx_sb, gsrc = cpa, idxA_sb, gsrcA
                    else:
                        cp, idx_sb, gsrc = cpb, idxB_sb, gsrcB
                    cols = cp * CHUNK // 16
                    nidx = GTI * cp * CHUNK
                    if "gather" in ablate:
                        key = ("Xdummy", li, which)
                        if key not in dummy_x:
                            Xd = cpool.tile(
                                [128, GTI * cp, 256], dt.float8e3, tag=f"Xd{li}{which}"
                            )
                            nc.vector.memset(Xd[:], 0.0)
                            dummy_x[key] = Xd
                        return dummy_x[key]
                    X = x8pool.tile([128, GTI * cp, 256], dt.float8e3, tag="X8")
                    src_ap = gsrc.ap()
                    if not fp8:
                        src_ap = src_ap.bitcast(dt.float8e3)
                    nc.gpsimd.dma_gather(
                        X[:],
                        src_ap,
                        idx_sb[:, g * GTI * cols : (g + 1) * GTI * cols],
                        nidx,
                        nidx,
                        256,
                        single_packet=False,
                        queue_num=g % NQUEUES,
                    )
                    return X

                def xv(X, idx):
                    """Matmul rhs view of gathered row `idx`."""
                    if fp8:
                        return X[:, idx, :fo]
                    return X[:, idx, :].bitcast(dt.bfloat16)[:, :fo]

                def self_out(li, t, X, tl, rpa, b_prev2, hT_out):
                    """k==0: out = relu(Z0 + bias + 0.5*(2 L b1) - b2)."""
                    zp = zpool.tile([128, HID], dt.float32, tag="z")
                    dense_tile(zp, hT_in, li, 0, t, True)
                    z_sb = spool.tile([128, HID], dt.bfloat16, tag="zsb")
                    nc.vector.tensor_copy(z_sb[:, :fo], zp[:, :fo])
                    rp = rpool.tile([128, HID], dt.float32, tag="r")
                    nc.tensor.matmul(
                        rp[:, :fo],
                        ident_sb[:],
                        rpa[:, t, :fo],
                        start=True,
                        stop=("sparse" in ablate),
                    )
                    for j in range(cpb):
                        if "sparse" in ablate:
                            break
                        e0 = (t * cpb + j) * CHUNK
                        nc.tensor.matmul(
                            rp[:, :fo],
                            sblkB_sb[:, e0 : e0 + CHUNK],
                            xv(X, tl * cpb + j),
                            start=False,
                            stop=(j == cpb - 1),
                        )
                    # out = relu(Z0L + 0.5*P2 - b2)
                    a1 = spool.tile([128, HID], dt.bfloat16, tag="a1")
                    nc.vector.tensor_scalar_mul(a1[:, :fo], rp[:, :fo], 0.5)
                    a2 = spool.tile([128, HID], dt.bfloat16, tag="ttmp")
                    nc.vector.tensor_sub(
                        a2[:, :fo], z_sb[:, :fo], b_prev2[:, t, :fo]
                    )
                    if li < 2:
                        h = spool.tile([128, HID], dt.bfloat16, tag="h")
                        nc.vector.tensor_add(h[:, :fo], a1[:, :fo], a2[:, :fo])
                        nc.vector.tensor_relu(h[:, :fo], h[:, :fo])
                        for c2 in range(fo // 128):
                            tp = tpool.tile([128, 128], dt.bfloat16, tag="tp")
                            nc.tensor.transpose(
                                tp[:],
                                h[:, c2 * 128 : (c2 + 1) * 128],
                                ident_sb[:],
                            )
                            nc.vector.tensor_copy(
                                hT_out[:, c2, t * 128 : (t + 1) * 128], tp[:]
                            )
                    else:
                        hf = spool.tile([128, F2REAL], dt.float32, tag="hf")
                        nc.vector.tensor_add(hf[:], a1[:, :fo], a2[:, :fo])
                        nc.vector.tensor_relu(hf[:], hf[:])
                        nc.sync.dma_start(
                            out_d.ap()[t * 128 : (t + 1) * 128, :], hf[:]
                        )

                # --- b4 = Z4, straight to bounce + SBUF state ---
                b4 = bpool.tile([128, TPC, HID], dt.bfloat16, tag="bst")
                with nc.named_scope(f"b4-L{li}"):
                    for t in range(TPC):
                        zp = zpool.tile([128, HID], dt.float32, tag="z")
                        dense_tile(zp, hT_in, li, 4, t, False)
                        nc.vector.tensor_copy(b4[:, t, :fo], zp[:, :fo])
                        bounce_tile(t, b4[:, t, :fo])
                        if t == HALF - 1:
                            ag_half(bnA, gfA, wdA)
                ag_half(bnB, gfB, wdB)

                b_prev2 = None  # b_{k+2}
                b_prev1 = b4  # b_{k+1} (already exchanged)
                for kth in (3, 2, 1, 0):
                    is_final = kth == 0
                    b_new = (
                        None
                        if is_final
                        else bpool.tile([128, TPC, HID], dt.bfloat16, tag="bst")
                    )
                    # ---- A phase: half-A sparse matmuls, spilled to SBUF ----
                    rpa = rapool.tile([128, TPC, HID], dt.bfloat16, tag="rpa")
                    with nc.named_scope(f"sparseA-L{li}k{kth}"):
                        for g in range(TPC // GTI):
                            if "sparse" in ablate:
                                break
                            X = gather_group(g, 0)
                            for tl in range(GTI):
                                t = g * GTI + tl
                                rp = rpool.tile([128, HID], dt.float32, tag="r")
                                for j in range(cpa):
                                    e0 = (t * cpa + j) * CHUNK
                                    nc.tensor.matmul(
                                        rp[:, :fo],
                                        sblkA_sb[:, e0 : e0 + CHUNK],
                                        xv(X, tl * cpa + j),
                                        start=(j == 0),
                                        stop=(j == cpa - 1),
                                    )
                                nc.vector.tensor_copy(rpa[:, t, :fo], rp[:, :fo])
                    # ---- B phase: dense Z + half-A inject + half-B sparse,
                    # all fused into one PSUM accumulation per tile ----
                    with nc.named_scope(f"sparseB-L{li}k{kth}"):
                        for g in range(TPC // GTI):
                            X = gather_group(g, 1)
                            for tl in range(GTI):
                                t = g * GTI + tl
                                if is_final:
                                    self_out(li, t, X, tl, rpa, b_prev2, hT_out)
                                    continue
                                rp = rpool.tile([128, HID], dt.float32, tag="r")
                                dense_tile(rp, hT_in, li, kth, t, False, stop=False)
                                nc.tensor.matmul(
                                    rp[:, :fo],
                                    ident_sb[:],
                                    rpa[:, t, :fo],
                                    start=False,
                                    stop=("sparse" in ablate),
                                )
                                for j in range(cpb):
                                    if "sparse" in ablate:
                                        break
                                    e0 = (t * cpb + j) * CHUNK
                                    nc.tensor.matmul(
                                        rp[:, :fo],
                                        sblkB_sb[:, e0 : e0 + CHUNK],
                                        xv(X, tl * cpb + j),
                                        start=False,
                                        stop=(j == cpb - 1),
                                    )
                                if kth == 3:
                                    nc.vector.tensor_copy(
                                        b_new[:, t, :fo], rp[:, :fo]
                                    )
                                else:
                                    nc.vector.tensor_sub(
                                        b_new[:, t, :fo],
                                        rp[:, :fo],
                                        b_prev2[:, t, :fo],
                                    )
                                bounce_tile(t, b_new[:, t, :fo])
                                if t == HALF - 1:
                                    ag_half(bnA, gfA, wdA)
                    if not is_final:
                        ag_half(bnB, gfB, wdB)
                        b_prev2 = b_prev1
                        b_prev1 = b_new

            for _ in range(reps):
                hT1 = hpool.tile([128, HID // 128, NROWS], dt.bfloat16, tag="hTn")
                run_layer(0, hT0, hT1)
                hT2 = hpool.tile([128, HID // 128, NROWS], dt.bfloat16, tag="hTn")
                run_layer(1, hT1, hT2)
                run_layer(2, hT2, None)

    nc.compile()
    return nc


# ----------------------------------------------------------------- runner ---

_CACHE = {}


def _get_nc(cpa, cpb, reps=1, ablate=()):
    key = (cpa, cpb, reps, tuple(ablate))
    if key not in _CACHE:
        _CACHE[key] = build_bass(cpa, cpb, reps, ablate=ablate)
    return _CACHE[key]


def make_in_maps(inputs):
    x = np.asarray(inputs["x"], np.float32)
    edge_index = np.asarray(inputs["edge_index"])
    meta, cores = _build_all(edge_index)
    gslot = meta["gslot"]
    cpa, cpb = meta["cpa"], meta["cpb"]
    epad_a = TPC * cpa * CHUNK
    epad_b = TPC * cpb * CHUNK

    weights = _fuse_weights(inputs)
    ident = np.eye(128, dtype=BF16)

    x_slot = np.zeros((NCORES * NROWS, KPAD_IN), np.float32)
    x_slot[gslot, :F_IN] = x

    in_maps = []
    for c in range(NCORES):
        (idxA, svalA, dlocA), (idxB, svalB, dlocB) = cores[c]
        m = {
            "xT": np.ascontiguousarray(
                x_slot[c * NROWS : (c + 1) * NROWS].T
            ).astype(BF16),
            "idxA": _pack_idx(idxA, cpa * CHUNK),
            "idxB": _pack_idx(idxB, cpb * CHUNK),
            "sblkA": _build_sblocks(svalA, dlocA, epad_a),
            "sblkB": _build_sblocks(svalB, dlocB, epad_b),
            "ident": ident,
        }
        for li in range(3):
            wf, wrest, bias = weights[li]
            m[f"wf{li}"] = wf
            m[f"wr{li}"] = wrest
            m[f"bias{li}"] = bias
        in_maps.append(m)
    return in_maps, meta


def assemble_output(results, meta):
    slot_node = meta["slot_node"]
    out_slot = np.concatenate([r["out_sh"] for r in results], axis=0)
    out = np.zeros((N, BOT), np.float32)
    valid = slot_node >= 0
    out[slot_node[valid]] = out_slot[valid][:, :BOT]
    return out


def kernel(**inputs):
    from concourse import bass_utils

    in_maps, meta = make_in_maps(inputs)
    nc = _get_nc(meta["cpa"], meta["cpb"])
    res = bass_utils.run_bass_kernel_spmd(nc, in_maps, core_ids=list(range(NCORES)))
    return assemble_output(res.results, meta)



# revision 31
# speedup vs baseline: 1.5677x; 1.5677x over previous
"""ChebGNN encoder (3x ChebConv(K=5) + Linear skip + ReLU) on 8 Trainium2
NeuronCores.

Strategy
--------
* Nodes are sharded over the 8 cores (graph parallel). Each core owns
  TPC dest-tiles of 128 node slots (padded rows; a host-chosen permutation
  bin-packs nodes into tiles so every (core, tile) has the same number of
  128-edge chunks).
* Each Chebyshev layer is evaluated with the Clenshaw recursion
      b4 = Z4;  b_k = Z_k + 2 L b_{k+1} - b_{k+2};  out = Z0 + L b1 - b2
  where Z_k = h @ Wc[k] (Z0 additionally fuses the Linear skip + biases,
  via host-side weight fusion and a K=1 ones x bias matmul).
* The sparse propagation L @ b is computed per dest-tile as
      psum += S_j.T @ X_j
  with S_j a [128 edge, 128 dest] bf16 block holding 2*w_edge
  (host-precomputed from edge_index) and X_j = dma_gather of the 128
  source rows from the all-gathered state in HBM.
* The state exchange is a split AllGather: the shard's tiles 0-9 (half A)
  are exchanged as soon as they are produced, so the half-A transfer
  overlaps the tail of the producing stage and the half-B transfer overlaps
  the next stage's half-A sparse work. Edge chunks are grouped by source
  half on the host, so half-A sparse matmuls only need the half-A exchange.
* Layers 0/1 exchange the b-state in fp8-e3m4; the state is kept scaled
  (s0=1/2, s1=1/4, folded into the dense weights host-side) so it fits
  e3m4's +-15.5 range. The sparse matmul takes the fp8 gather output
  directly (bf16 S x fp8 X). Layer 2 exchanges 64-wide bf16 and widens to
  256B rows for the gather.
"""

import numpy as np
import ml_dtypes

BF16 = ml_dtypes.bfloat16

# ---------------------------------------------------------------- config ---

N = 20000
E = 320000
F_IN = 350
HID = 256
BOT = 64
K = 5
NCORES = 8
CHUNK = 128
TPC = 20                    # dest tiles per core
GTI = 1                     # dest tiles per gather group
HALF = TPC // 2             # tiles per exchange half
HROWS = HALF * CHUNK        # rows per half (1280)
NROWS = TPC * CHUNK         # padded rows per core (2560)
KPAD_IN = 384               # F_IN padded to 3*128
F2PAD = 128                 # layer-2 gather row width (256B rows)
F2REAL = 64                 # layer-2 compute width (BOT=64, no padding)

# Per-layer scale of the b-state space: layers 0/1 exchange b in fp8-e3m4
# (max 15.5), so b is kept scaled to fit: |b0|<12 -> s0=1/2, |b1|<51 -> s1=1/4.
# The scale is folded into the dense weights (W_li *= s_li/s_{li-1}), so the
# device-side recursion/prop is identical to the unscaled bf16 version.
SCALES = (0.5, 0.25, 1.0)


def _layer_dims(li):
    # (kpad = padded contraction dim, f_out = compute output width)
    return ((KPAD_IN, HID), (HID, HID), (HID, F2REAL))[li]


# ------------------------------------------------------ host preprocessing ---


def _edge_norm_host(edge_index):
    row = edge_index[0].astype(np.int64)
    col = edge_index[1].astype(np.int64)
    mask = row != col
    deg = np.bincount(row[mask], minlength=N).astype(np.float32)
    with np.errstate(divide="ignore"):
        dinv = np.where(deg > 0, 1.0 / np.sqrt(np.maximum(deg, 1e-12)), 0.0).astype(
            np.float32
        )
    w = (-dinv[row] * dinv[col]).astype(np.float32)
    w = np.where(mask, w, 0.0)
    return row, col, w, mask


def _build_all(edge_index):
    """Node permutation + per-core padded edge structure, split by source half.

    Returns (meta, cores): meta has gslot [N], slot_node, cpa, cpb; cores[c]
    has ((idxA, svalA, dlocA), (idxB, svalB, dlocB)).
    """
    row, col, w, mask = _edge_norm_host(edge_index)
    indeg = np.bincount(col[mask], minlength=N).astype(np.int64)

    nloc = N // NCORES
    order = np.argsort(-indeg, kind="stable")
    node_core = np.empty(N, dtype=np.int64)
    rounds = np.arange(N) // NCORES
    fwd = np.arange(N) % NCORES
    snake = np.where(rounds % 2 == 0, fwd, NCORES - 1 - fwd)
    node_core[order] = snake

    tile_of_node = np.empty(N, dtype=np.int64)
    slot_of_node = np.empty(N, dtype=np.int64)
    for c in range(NCORES):
        nodes_c = np.flatnonzero(node_core == c)
        assert len(nodes_c) == nloc
        heavy_first = nodes_c[np.argsort(-indeg[nodes_c], kind="stable")]
        loads = np.zeros(TPC, dtype=np.int64)
        counts = np.zeros(TPC, dtype=np.int64)
        for nd in heavy_first:
            cand = np.flatnonzero(counts < CHUNK)
            t = cand[np.argmin(loads[cand])]
            tile_of_node[nd] = t
            slot_of_node[nd] = counts[t]
            counts[t] += 1
            loads[t] += indeg[nd]

    # Rebalance tiles so per-(core, tile, half) in-edge counts stay <= 8
    # chunks: a node's (nA, nB) split depends on its SOURCES' tile halves,
    # which the repacking itself changes -> iterate a few rounds.
    er0, ec0 = row[mask], col[mask]
    for rnd in range(6):
        repack = rnd < 3
        src_is_b = (tile_of_node[er0] >= HALF).astype(np.int64)
        na = np.bincount(ec0[src_is_b == 0], minlength=N).astype(np.int64)
        nb = np.bincount(ec0[src_is_b == 1], minlength=N).astype(np.int64)
        for c in range(NCORES):
            nodes_c = np.flatnonzero(node_core == c)
            A = np.zeros(TPC, dtype=np.int64)
            B = np.zeros(TPC, dtype=np.int64)
            cnt_t = np.zeros(TPC, dtype=np.int64)
            tile_members = {(c, t): [] for t in range(TPC)}
            if repack:
                order_c = nodes_c[
                    np.argsort(-(na[nodes_c] + nb[nodes_c]), kind="stable")
                ]
                for nd in order_c:
                    free = cnt_t < CHUNK
                    pa = A + na[nd]
                    pb = B + nb[nd]
                    score = (
                        np.maximum(pa, pb)
                        + 1_000_000 * (pa > 8 * CHUNK)
                        + 1_000_000 * (pb > 8 * CHUNK)
                    )
                    score[~free] = np.iinfo(np.int64).max
                    t = int(np.argmin(score))
                    tile_of_node[nd] = t
                    tile_members[(c, t)].append(nd)
                    cnt_t[t] += 1
                    A[t] += na[nd]
                    B[t] += nb[nd]
            else:
                for nd in nodes_c:
                    t = int(tile_of_node[nd])
                    tile_members[(c, t)].append(nd)
                    cnt_t[t] += 1
                    A[t] += na[nd]
                    B[t] += nb[nd]
            # repair: move nodes out of >8-chunk (tile, half) bins
            for _rep in range(400):
                worst = max(range(TPC), key=lambda t: max(A[t], B[t]))
                wv = max(A[worst], B[worst])
                if wv <= 8 * CHUNK:
                    break
                hv = 0 if A[worst] >= B[worst] else 1
                members = tile_members[(c, worst)]
                key = na if hv == 0 else nb
                members.sort(key=lambda nd: -key[nd])
                moved = False
                for nd in members[:24]:
                    va, vb = na[nd], nb[nd]
                    ok = (
                        (cnt_t < CHUNK)
                        & (A + va <= 8 * CHUNK)
                        & (B + vb <= 8 * CHUNK)
                    )
                    ok[worst] = False
                    cand = np.flatnonzero(ok)
                    if len(cand):
                        t2 = int(cand[np.argmin(np.maximum(A, B)[cand])])
                        members.remove(nd)
                        tile_members[(c, t2)].append(nd)
                        tile_of_node[nd] = t2
                        cnt_t[worst] -= 1
                        cnt_t[t2] += 1
                        A[worst] -= va
                        B[worst] -= vb
                        A[t2] += va
                        B[t2] += vb
                        moved = True
                        break
                if not moved:
                    break

    # final slot assignment from tile membership
    for c in range(NCORES):
        for t in range(TPC):
            members = np.flatnonzero((node_core == c) & (tile_of_node == t))
            slot_of_node[members] = np.arange(len(members))

    gslot = node_core * NROWS + tile_of_node * CHUNK + slot_of_node
    slot_node = np.full(NCORES * NROWS, -1, dtype=np.int64)
    slot_node[gslot] = np.arange(N)

    er, ec, ew = row[mask], col[mask], w[mask]
    dest_core = node_core[ec]
    dest_tile = tile_of_node[ec]
    dest_slot = slot_of_node[ec]
    src_gslot = gslot[er]
    src_core = src_gslot // NROWS
    src_loc = src_gslot % NROWS
    src_half = src_loc >= HROWS  # False=A (tiles 0-9), True=B
    # row index within the half-exchange buffer [NCORES*HROWS, fo]
    src_hrow = src_core * HROWS + np.where(src_half, src_loc - HROWS, src_loc)

    # global chunk counts per half
    cnt = np.zeros((2, NCORES, TPC), dtype=np.int64)
    np.add.at(cnt, (src_half.astype(np.int64), dest_core, dest_tile), 1)
    cpa = int(np.ceil(cnt[0].max() / CHUNK))
    cpb = int(np.ceil(cnt[1].max() / CHUNK))
    # per-(half, tile) REAL chunk count: max over cores, so one SPMD program
    # can gather/matmul only the needed chunks (pads within the last real
    # chunk point at row 0 with S weight 0).
    cpj = tuple(
        tuple(int(np.ceil(cnt[h, :, t].max() / CHUNK)) for t in range(TPC))
        for h in (0, 1)
    )

    cores = []
    for c in range(NCORES):
        m = dest_core == c
        halves = []
        for h, cp in ((0, cpa), (1, cpb)):
            mh = m & (src_half == (h == 1))
            t_arr = dest_tile[mh]
            s_arr = dest_slot[mh]
            src_arr = src_hrow[mh]
            w_arr = ew[mh]
            o = np.argsort(t_arr, kind="stable")
            t_arr, s_arr, src_arr, w_arr = t_arr[o], s_arr[o], src_arr[o], w_arr[o]
            epad = TPC * cp * CHUNK
            idx = np.zeros(epad, dtype=np.int16)
            sval = np.zeros(epad, dtype=np.float32)
            dloc = np.zeros(epad, dtype=np.int16)
            starts = np.searchsorted(t_arr, np.arange(TPC))
            ends = np.searchsorted(t_arr, np.arange(TPC) + 1)
            for t in range(TPC):
                a, b = starts[t], ends[t]
                base = t * cp * CHUNK
                idx[base : base + b - a] = src_arr[a:b]
                sval[base : base + b - a] = 2.0 * w_arr[a:b]
                dloc[base : base + b - a] = s_arr[a:b]
            halves.append((idx, sval, dloc))
        cores.append(halves)

    meta = dict(gslot=gslot, slot_node=slot_node, cpa=cpa, cpb=cpb, cpj=cpj)
    return meta, cores


def _pack_idx(idx, piece):
    """dma_gather index layout: flat gathered position i within a piece reads
    idx_sb[i % 16, i // 16] (column-major wrap over 16 partitions), pieces
    side by side along the free dim, replicated to 128 partitions."""
    epad = idx.shape[0]
    blocks = [
        np.ascontiguousarray(idx[g * piece : (g + 1) * piece].reshape(piece // 16, 16).T)
        for g in range(epad // piece)
    ]
    arr = np.concatenate(blocks, axis=1)
    return np.ascontiguousarray(np.tile(arr, (8, 1))).astype(np.int16)


def _build_sblocks(sval, dloc, epad):
    s = np.zeros((CHUNK, epad), dtype=np.float32)
    flat = np.arange(epad)
    j = flat // CHUNK
    kk = flat % CHUNK
    s[kk, j * CHUNK + dloc.astype(np.int64)] = sval
    return s.astype(BF16)


def _fuse_weights(inputs):
    """Per layer: (wf [kpad, f_out] = Wc[0]+Wl (+0 pad), wrest [4, kpad, f_out],
    bias [1, f_out] = bc+bl), all bf16, scaled by s_li/s_{li-1}."""
    out = []
    for li in range(3):
        kpad, f_out = _layer_dims(li)
        gain = SCALES[li] / (SCALES[li - 1] if li > 0 else 1.0)
        Wc = np.asarray(inputs[f"Wc{li}"], np.float32) * gain
        Wl = np.asarray(inputs[f"Wl{li}"], np.float32) * gain
        bc = np.asarray(inputs[f"bc{li}"], np.float32) * SCALES[li]
        bl = np.asarray(inputs[f"bl{li}"], np.float32) * SCALES[li]
        f_in, f_real = Wc.shape[1], Wc.shape[2]
        wf = np.zeros((kpad, f_out), np.float32)
        wf[:f_in, :f_real] = Wc[0] + Wl
        wrest = np.zeros((K - 1, kpad, f_out), np.float32)
        for k in range(1, K):
            wrest[k - 1, :f_in, :f_real] = Wc[k]
        bias = np.zeros((1, f_out), np.float32)
        bias[0, :f_real] = bc + bl
        out.append((wf.astype(BF16), wrest.astype(BF16), bias.astype(BF16)))
    return out


# ------------------------------------------------------------- device side ---


NQUEUES = 4


def build_bass(cpa, cpb, reps=1, ablate=(), cpj=None):
    if cpj is None:
        cpj = ((cpa,) * TPC, (cpb,) * TPC)
    import concourse.bacc as bacc
    import concourse.bass as bass
    import concourse.mybir as mybir
    import concourse.tile as tile
    from concourse import library_config

    dt = mybir.dt
    epad_a = TPC * cpa * CHUNK
    epad_b = TPC * cpb * CHUNK

    nc = bacc.Bacc(
        "TRN2",
        target_bir_lowering=False,
        debug=False,
        num_devices=NCORES,
        num_swdge_queues=NQUEUES,
    )

    # ---- I/O ----
    xT_d = nc.dram_tensor("xT", [KPAD_IN, NROWS], dt.bfloat16, kind="ExternalInput")
    idxA_d = nc.dram_tensor("idxA", [128, epad_a // 16], dt.int16, kind="ExternalInput")
    idxB_d = nc.dram_tensor("idxB", [128, epad_b // 16], dt.int16, kind="ExternalInput")
    sblkA_d = nc.dram_tensor("sblkA", [128, epad_a], dt.bfloat16, kind="ExternalInput")
    sblkB_d = nc.dram_tensor("sblkB", [128, epad_b], dt.bfloat16, kind="ExternalInput")
    ident_d = nc.dram_tensor("ident", [128, 128], dt.bfloat16, kind="ExternalInput")
    w_d = []
    for li in range(3):
        kpad, f_out = _layer_dims(li)
        w_d.append(
            (
                nc.dram_tensor(f"wf{li}", [kpad, f_out], dt.bfloat16, kind="ExternalInput"),
                nc.dram_tensor(
                    f"wr{li}", [K - 1, kpad, f_out], dt.bfloat16, kind="ExternalInput"
                ),
                nc.dram_tensor(f"bias{li}", [1, f_out], dt.bfloat16, kind="ExternalInput"),
            )
        )
    out_d = nc.dram_tensor("out_sh", [NROWS, F2REAL], dt.float32, kind="ExternalOutput")

    # internal DRAM for the split state exchange
    bnA8 = nc.dram_tensor("bnA8", [HROWS, HID], dt.float8e3, kind="Internal")
    bnB8 = nc.dram_tensor("bnB8", [HROWS, HID], dt.float8e3, kind="Internal")
    bfA8 = nc.dram_tensor(
        "bfA8", [NCORES * HROWS, HID], dt.float8e3, kind="Internal", addr_space="Shared"
    )
    bfB8 = nc.dram_tensor(
        "bfB8", [NCORES * HROWS, HID], dt.float8e3, kind="Internal", addr_space="Shared"
    )
    bnA2 = nc.dram_tensor("bnA2", [HROWS, F2REAL], dt.bfloat16, kind="Internal")
    bnB2 = nc.dram_tensor("bnB2", [HROWS, F2REAL], dt.bfloat16, kind="Internal")
    bfA264 = nc.dram_tensor(
        "bfA264", [NCORES * HROWS, F2REAL], dt.bfloat16, kind="Internal",
        addr_space="Shared",
    )
    bfB264 = nc.dram_tensor(
        "bfB264", [NCORES * HROWS, F2REAL], dt.bfloat16, kind="Internal",
        addr_space="Shared",
    )
    bfA2 = nc.dram_tensor("bfA2", [NCORES * HROWS, F2PAD], dt.bfloat16, kind="Internal")
    bfB2 = nc.dram_tensor("bfB2", [NCORES * HROWS, F2PAD], dt.bfloat16, kind="Internal")

    groups = [list(range(NCORES))]

    with tile.TileContext(nc) as tc:
        with (
            tc.tile_pool(name="const", bufs=1) as cpool,
            tc.tile_pool(name="hT", bufs=2) as hpool,
            tc.tile_pool(name="bstate", bufs=3) as bpool,
            tc.tile_pool(name="rpa", bufs=1) as rapool,
            tc.tile_pool(name="xbuf", bufs=2) as xpool,
            tc.tile_pool(name="x8buf", bufs=8) as x8pool,
            tc.tile_pool(name="small", bufs=1) as spool,
            tc.tile_pool(name="zpsum", bufs=2, space="PSUM") as zpool,
            tc.tile_pool(name="rpsum", bufs=3, space="PSUM") as rpool,
            tc.tile_pool(name="tpsum", bufs=2, space="PSUM") as tpool,
        ):
            nc.gpsimd.load_library(library_config.mlp)
            dummy_x = {}

            # ---- resident loads (hT0 + weights first: they gate layer-0 Z4) ----
            hT0 = cpool.tile([128, KPAD_IN // 128, NROWS], dt.bfloat16, tag="hT0")
            nc.sync.dma_start(hT0[:], xT_d.ap().rearrange("(c p) n -> p c n", p=128))

            w_sb = []
            for li in range(3):
                kpad, f_out = _layer_dims(li)
                kc = kpad // 128
                wf_sb = cpool.tile([128, kc, f_out], dt.bfloat16, tag=f"wf{li}")
                nc.sync.dma_start(
                    wf_sb[:], w_d[li][0].ap().rearrange("(c p) f -> p c f", p=128)
                )
                wr_sb = cpool.tile([128, K - 1, kc, f_out], dt.bfloat16, tag=f"wr{li}")
                nc.sync.dma_start(
                    wr_sb[:], w_d[li][1].ap().rearrange("k (c p) f -> p k c f", p=128)
                )
                bias_sb = cpool.tile([1, f_out], dt.bfloat16, tag=f"bias{li}")
                nc.sync.dma_start(bias_sb[:], w_d[li][2].ap())
                w_sb.append((wf_sb, wr_sb, bias_sb))

            sblkA_sb = cpool.tile([128, epad_a], dt.bfloat16, tag="sblkA")
            nc.sync.dma_start(sblkA_sb[:], sblkA_d.ap())
            sblkB_sb = cpool.tile([128, epad_b], dt.bfloat16, tag="sblkB")
            nc.sync.dma_start(sblkB_sb[:], sblkB_d.ap())
            idxA_sb = cpool.tile([128, epad_a // 16], dt.int16, tag="idxA")
            nc.sync.dma_start(idxA_sb[:], idxA_d.ap())
            idxB_sb = cpool.tile([128, epad_b // 16], dt.int16, tag="idxB")
            nc.sync.dma_start(idxB_sb[:], idxB_d.ap())
            ident_sb = cpool.tile([128, 128], dt.bfloat16, tag="ident")
            nc.sync.dma_start(ident_sb[:], ident_d.ap())
            ones_sb = cpool.tile([1, 128], dt.bfloat16, tag="ones")
            nc.vector.memset(ones_sb[:], 1.0)

            def dense_tile(zp, hT_in, li, widx, t, with_bias, stop=True):
                """psum[128 nodes, f_out] = h_tile @ W  (+ ones x bias)."""
                kpad, f_out = _layer_dims(li)
                kc = kpad // 128
                wf_sb, wr_sb, bias_sb = w_sb[li]
                zv = zp[:, :f_out]
                for c in range(kc):
                    lhsT = hT_in[:, c, t * 128 : (t + 1) * 128]
                    rhs = wf_sb[:, c, :] if widx == 0 else wr_sb[:, widx - 1, c, :]
                    nc.tensor.matmul(
                        zv,
                        lhsT,
                        rhs,
                        start=(c == 0),
                        stop=(stop and c == kc - 1 and not with_bias),
                    )
                if with_bias:
                    nc.tensor.matmul(
                        zv, ones_sb[:1, :], bias_sb[:1, :], start=False, stop=stop
                    )

            def ag_half(bounce, bfull, widen_to=None):
                if "coll" in ablate:
                    return
                with nc.named_scope("AG"):
                    nc.gpsimd.collective_compute(
                        "AllGather",
                        mybir.AluOpType.bypass,
                        replica_groups=groups,
                        ins=[bounce.ap().opt()],
                        outs=[bfull.ap().opt()],
                    )
                    if widen_to is not None:
                        nc.sync.dma_start(widen_to.ap()[:, :F2REAL], bfull.ap())

            def run_layer(li, hT_in, hT_out):
                kpad, f_out = _layer_dims(li)
                fo = f_out
                fp8 = li < 2
                fg = F2PAD if li == 2 else fo  # gather row width (256B granule)
                if fp8:
                    bnA, bnB, gfA, gfB = bnA8, bnB8, bfA8, bfB8
                    wdA = wdB = None
                    gsrcA, gsrcB = bfA8, bfB8
                else:
                    bnA, bnB, gfA, gfB = bnA2, bnB2, bfA264, bfB264
                    wdA, wdB = bfA2, bfB2
                    gsrcA, gsrcB = bfA2, bfB2

                def bounce_tile(t, src_tile):
                    bn = bnA if t < HALF else bnB
                    th = t % HALF
                    if fp8:
                        st = spool.tile([128, HID], dt.float8e3, tag="st8")
                        nc.vector.tensor_copy(st[:, :fo], src_tile)
                        nc.sync.dma_start(
                            bn.ap()[th * 128 : (th + 1) * 128, :fo], st[:, :fo]
                        )
                    else:
                        nc.sync.dma_start(
                            bn.ap()[th * 128 : (th + 1) * 128, :], src_tile
                        )

                def gather_group(g, which):
                    """One dma_gather for tile g's real chunks (cpj[which][g]).

                    X is always an fp8-typed [128, cp, 256] byte buffer
                    (256B per gathered row); layer 2's bf16 rows are bitcast
                    back at the matmul site."""
                    if which == 0:
                        cp, idx_sb, gsrc = cpa, idxA_sb, gsrcA
                    else:
                        cp, idx_sb, gsrc = cpb, idxB_sb, gsrcB
                    cpjv = cpj[which][g]
                    nidx = cpjv * CHUNK
                    if "gather" in ablate:
                        key = ("Xdummy", li, which)
                        if key not in dummy_x:
                            Xd = cpool.tile(
                                [128, cp, 256], dt.float8e3, tag=f"Xd{li}{which}"
                            )
                            nc.vector.memset(Xd[:], 0.0)
                            dummy_x[key] = Xd
                        return dummy_x[key]
                    X = x8pool.tile([128, cp, 256], dt.float8e3, tag="X8")
                    src_ap = gsrc.ap()
                    if not fp8:
                        src_ap = src_ap.bitcast(dt.float8e3)
                    col0 = g * cp * CHUNK // 16
                    nc.gpsimd.dma_gather(
                        X[:, :cpjv, :],
                        src_ap,
                        idx_sb[:, col0 : col0 + nidx // 16],
                        nidx,
                        nidx,
                        256,
                        single_packet=False,
                        queue_num=g % NQUEUES,
                    )
                    return X

                def xv(X, idx):
                    """Matmul rhs view of gathered row `idx`."""
                    if fp8:
                        return X[:, idx, :fo]
                    return X[:, idx, :].bitcast(dt.bfloat16)[:, :fo]

                def self_out(li, t, X, tl, rpa, b_prev2, hT_out):
                    """k==0: out = relu(Z0 + bias + 0.5*(2 L b1) - b2)."""
                    zp = zpool.tile([128, HID], dt.float32, tag="z")
                    dense_tile(zp, hT_in, li, 0, t, True)
                    z_sb = spool.tile([128, HID], dt.bfloat16, tag="zsb")
                    nc.vector.tensor_copy(z_sb[:, :fo], zp[:, :fo])
                    rp = rpool.tile([128, HID], dt.float32, tag="r")
                    nc.tensor.matmul(
                        rp[:, :fo],
                        ident_sb[:],
                        rpa[:, t, :fo],
                        start=True,
                        stop=("sparse" in ablate),
                    )
                    nj = cpj[1][t]
                    for j in range(nj):
                        if "sparse" in ablate:
                            break
                        e0 = (t * cpb + j) * CHUNK
                        nc.tensor.matmul(
                            rp[:, :fo],
                            sblkB_sb[:, e0 : e0 + CHUNK],
                            xv(X, tl * cpb + j),
                            start=False,
                            stop=(j == nj - 1),
                        )
                    # out = relu(Z0L + 0.5*P2 - b2)
                    a1 = spool.tile([128, HID], dt.bfloat16, tag="a1")
                    nc.vector.tensor_scalar_mul(a1[:, :fo], rp[:, :fo], 0.5)
                    a2 = spool.tile([128, HID], dt.bfloat16, tag="ttmp")
                    nc.vector.tensor_sub(
                        a2[:, :fo], z_sb[:, :fo], b_prev2[:, t, :fo]
                    )
                    if li < 2:
                        h = spool.tile([128, HID], dt.bfloat16, tag="h")
                        nc.vector.tensor_add(h[:, :fo], a1[:, :fo], a2[:, :fo])
                        nc.vector.tensor_relu(h[:, :fo], h[:, :fo])
                        for c2 in range(fo // 128):
                            tp = tpool.tile([128, 128], dt.bfloat16, tag="tp")
                            nc.tensor.transpose(
                                tp[:],
                                h[:, c2 * 128 : (c2 + 1) * 128],
                                ident_sb[:],
                            )
                            nc.vector.tensor_copy(
                                hT_out[:, c2, t * 128 : (t + 1) * 128], tp[:]
                            )
                    else:
                        hf = spool.tile([128, F2REAL], dt.float32, tag="hf")
                        nc.vector.tensor_add(hf[:], a1[:, :fo], a2[:, :fo])
                        nc.vector.tensor_relu(hf[:], hf[:])
                        nc.sync.dma_start(
                            out_d.ap()[t * 128 : (t + 1) * 128, :], hf[:]
                        )

                # --- b4 = Z4, straight to bounce + SBUF state ---
                b4 = bpool.tile([128, TPC, HID], dt.bfloat16, tag="bst")
                with nc.named_scope(f"b4-L{li}"):
                    for t in range(TPC):
                        zp = zpool.tile([128, HID], dt.float32, tag="z")
                        dense_tile(zp, hT_in, li, 4, t, False)
                        nc.vector.tensor_copy(b4[:, t, :fo], zp[:, :fo])
                        bounce_tile(t, b4[:, t, :fo])
                        if t == HALF - 1:
                            ag_half(bnA, gfA, wdA)
                ag_half(bnB, gfB, wdB)

                b_prev2 = None  # b_{k+2}
                b_prev1 = b4  # b_{k+1} (already exchanged)
                for kth in (3, 2, 1, 0):
                    is_final = kth == 0
                    b_new = (
                        None
                        if is_final
                        else bpool.tile([128, TPC, HID], dt.bfloat16, tag="bst")
                    )
                    # ---- A phase: half-A sparse matmuls, spilled to SBUF ----
                    rpa = rapool.tile([128, TPC, HID], dt.bfloat16, tag="rpa")
                    with nc.named_scope(f"sparseA-L{li}k{kth}"):
                        for g in range(TPC // GTI):
                            if "sparse" in ablate:
                                break
                            X = gather_group(g, 0)
                            for tl in range(GTI):
                                t = g * GTI + tl
                                rp = rpool.tile([128, HID], dt.float32, tag="r")
                                nj = cpj[0][t]
                                for j in range(nj):
                                    e0 = (t * cpa + j) * CHUNK
                                    nc.tensor.matmul(
                                        rp[:, :fo],
                                        sblkA_sb[:, e0 : e0 + CHUNK],
                                        xv(X, tl * cpa + j),
                                        start=(j == 0),
                                        stop=(j == nj - 1),
                                    )
                                nc.vector.tensor_copy(rpa[:, t, :fo], rp[:, :fo])
                    # ---- B phase: dense Z + half-A inject + half-B sparse,
                    # all fused into one PSUM accumulation per tile ----
                    with nc.named_scope(f"sparseB-L{li}k{kth}"):
                        for g in range(TPC // GTI):
                            X = gather_group(g, 1)
                            for tl in range(GTI):
                                t = g * GTI + tl
                                if is_final:
                                    self_out(li, t, X, tl, rpa, b_prev2, hT_out)
                                    continue
                                rp = rpool.tile([128, HID], dt.float32, tag="r")
                                dense_tile(rp, hT_in, li, kth, t, False, stop=False)
                                nc.tensor.matmul(
                                    rp[:, :fo],
                                    ident_sb[:],
                                    rpa[:, t, :fo],
                                    start=False,
                                    stop=("sparse" in ablate),
                                )
                                nj = cpj[1][t]
                                for j in range(nj):
                                    if "sparse" in ablate:
                                        break
                                    e0 = (t * cpb + j) * CHUNK
                                    nc.tensor.matmul(
                                        rp[:, :fo],
                                        sblkB_sb[:, e0 : e0 + CHUNK],
                                        xv(X, tl * cpb + j),
                                        start=False,
                                        stop=(j == nj - 1),
                                    )
                                if kth == 3:
                                    nc.vector.tensor_copy(
                                        b_new[:, t, :fo], rp[:, :fo]
                                    )
                                else:
                                    nc.vector.tensor_sub(
                                        b_new[:, t, :fo],
                                        rp[:, :fo],
                                        b_prev2[:, t, :fo],
                                    )
                                bounce_tile(t, b_new[:, t, :fo])
                                if t == HALF - 1:
                                    ag_half(bnA, gfA, wdA)
                    if not is_final:
                        ag_half(bnB, gfB, wdB)
                        b_prev2 = b_prev1
                        b_prev1 = b_new

            for _ in range(reps):
                hT1 = hpool.tile([128, HID // 128, NROWS], dt.bfloat16, tag="hTn")
                run_layer(0, hT0, hT1)
                hT2 = hpool.tile([128, HID // 128, NROWS], dt.bfloat16, tag="hTn")
                run_layer(1, hT1, hT2)
                run_layer(2, hT2, None)

    nc.compile()
    return nc


# ----------------------------------------------------------------- runner ---

_CACHE = {}


def _get_nc(cpa, cpb, reps=1, ablate=(), cpj=None):
    key = (cpa, cpb, reps, tuple(ablate), cpj)
    if key not in _CACHE:
        _CACHE[key] = build_bass(cpa, cpb, reps, ablate=ablate, cpj=cpj)
    return _CACHE[key]


def make_in_maps(inputs):
    x = np.asarray(inputs["x"], np.float32)
    edge_index = np.asarray(inputs["edge_index"])
    meta, cores = _build_all(edge_index)
    gslot = meta["gslot"]
    cpa, cpb = meta["cpa"], meta["cpb"]
    epad_a = TPC * cpa * CHUNK
    epad_b = TPC * cpb * CHUNK

    weights = _fuse_weights(inputs)
    ident = np.eye(128, dtype=BF16)

    x_slot = np.zeros((NCORES * NROWS, KPAD_IN), np.float32)
    x_slot[gslot, :F_IN] = x

    in_maps = []
    for c in range(NCORES):
        (idxA, svalA, dlocA), (idxB, svalB, dlocB) = cores[c]
        m = {
            "xT": np.ascontiguousarray(
                x_slot[c * NROWS : (c + 1) * NROWS].T
            ).astype(BF16),
            "idxA": _pack_idx(idxA, cpa * CHUNK),
            "idxB": _pack_idx(idxB, cpb * CHUNK),
            "sblkA": _build_sblocks(svalA, dlocA, epad_a),
            "sblkB": _build_sblocks(svalB, dlocB, epad_b),
            "ident": ident,
        }
        for li in range(3):
            wf, wrest, bias = weights[li]
            m[f"wf{li}"] = wf
            m[f"wr{li}"] = wrest
            m[f"bias{li}"] = bias
        in_maps.append(m)
    return in_maps, meta


def assemble_output(results, meta):
    slot_node = meta["slot_node"]
    out_slot = np.concatenate([r["out_sh"] for r in results], axis=0)
    out = np.zeros((N, BOT), np.float32)
    valid = slot_node >= 0
    out[slot_node[valid]] = out_slot[valid][:, :BOT]
    return out


def kernel(**inputs):
    from concourse import bass_utils

    in_maps, meta = make_in_maps(inputs)
    nc = _get_nc(meta["cpa"], meta["cpb"], cpj=meta["cpj"])
    res = bass_utils.run_bass_kernel_spmd(nc, in_maps, core_ids=list(range(NCORES)))
    return assemble_output(res.results, meta)



# revision 32
# speedup vs baseline: 2.7539x; 1.7566x over previous
"""ChebGNN encoder (3x ChebConv(K=5) + Linear skip + ReLU) on 8 Trainium2
NeuronCores.

Strategy
--------
* Nodes are sharded over the 8 cores (graph parallel). Each core owns
  TPC dest-tiles of 128 node slots (padded rows; a host-chosen permutation
  bin-packs nodes into tiles so every (core, tile) has the same number of
  128-edge chunks).
* Each Chebyshev layer is evaluated with the Clenshaw recursion
      b4 = Z4;  b_k = Z_k + 2 L b_{k+1} - b_{k+2};  out = Z0 + L b1 - b2
  where Z_k = h @ Wc[k] (Z0 additionally fuses the Linear skip + biases,
  via host-side weight fusion and a K=1 ones x bias matmul).
* The sparse propagation L @ b is computed per dest-tile as
      psum += S_j.T @ X_j
  with S_j a [128 edge, 128 dest] bf16 block holding 2*w_edge
  (host-precomputed from edge_index) and X_j = dma_gather of the 128
  source rows from the all-gathered state in HBM.
* The state exchange is a split AllGather: the shard's tiles 0-9 (half A)
  are exchanged as soon as they are produced, so the half-A transfer
  overlaps the tail of the producing stage and the half-B transfer overlaps
  the next stage's half-A sparse work. Edge chunks are grouped by source
  half on the host, so half-A sparse matmuls only need the half-A exchange.
* Layers 0/1 exchange the b-state in fp8-e3m4; the state is kept scaled
  (s0=1/2, s1=1/4, folded into the dense weights host-side) so it fits
  e3m4's +-15.5 range. The sparse matmul takes the fp8 gather output
  directly (bf16 S x fp8 X). Layer 2 exchanges 64-wide bf16 and widens to
  256B rows for the gather.
"""

import numpy as np
import ml_dtypes

BF16 = ml_dtypes.bfloat16

# ---------------------------------------------------------------- config ---

N = 20000
E = 320000
F_IN = 350
HID = 256
BOT = 64
K = 5
NCORES = 8
CHUNK = 128
TPC = 20                    # dest tiles per core
GTI = 1                     # dest tiles per gather group
HALF = TPC // 2             # tiles per exchange half
HROWS = HALF * CHUNK        # rows per half (1280)
NROWS = TPC * CHUNK         # padded rows per core (2560)
KPAD_IN = 384               # F_IN padded to 3*128
F2PAD = 128                 # layer-2 gather row width (256B rows)
F2REAL = 64                 # layer-2 compute width (BOT=64, no padding)

# Per-layer scale of the b-state space: layers 0/1 exchange b in fp8-e3m4
# (max 15.5), so b is kept scaled to fit: |b0|<12 -> s0=1/2, |b1|<51 -> s1=1/4.
# The scale is folded into the dense weights (W_li *= s_li/s_{li-1}), so the
# device-side recursion/prop is identical to the unscaled bf16 version.
SCALES = (0.5, 0.25, 1.0)


def _layer_dims(li):
    # (kpad = padded contraction dim, f_out = compute output width)
    return ((KPAD_IN, HID), (HID, HID), (HID, F2REAL))[li]


# ------------------------------------------------------ host preprocessing ---


def _edge_norm_host(edge_index):
    row = edge_index[0].astype(np.int64)
    col = edge_index[1].astype(np.int64)
    mask = row != col
    deg = np.bincount(row[mask], minlength=N).astype(np.float32)
    with np.errstate(divide="ignore"):
        dinv = np.where(deg > 0, 1.0 / np.sqrt(np.maximum(deg, 1e-12)), 0.0).astype(
            np.float32
        )
    w = (-dinv[row] * dinv[col]).astype(np.float32)
    w = np.where(mask, w, 0.0)
    return row, col, w, mask


def _build_all(edge_index):
    """Node permutation + per-core padded edge structure, split by source half.

    Returns (meta, cores): meta has gslot [N], slot_node, cpa, cpb; cores[c]
    has ((idxA, svalA, dlocA), (idxB, svalB, dlocB)).
    """
    row, col, w, mask = _edge_norm_host(edge_index)
    indeg = np.bincount(col[mask], minlength=N).astype(np.int64)

    nloc = N // NCORES
    order = np.argsort(-indeg, kind="stable")
    node_core = np.empty(N, dtype=np.int64)
    rounds = np.arange(N) // NCORES
    fwd = np.arange(N) % NCORES
    snake = np.where(rounds % 2 == 0, fwd, NCORES - 1 - fwd)
    node_core[order] = snake

    tile_of_node = np.empty(N, dtype=np.int64)
    slot_of_node = np.empty(N, dtype=np.int64)
    for c in range(NCORES):
        nodes_c = np.flatnonzero(node_core == c)
        assert len(nodes_c) == nloc
        heavy_first = nodes_c[np.argsort(-indeg[nodes_c], kind="stable")]
        loads = np.zeros(TPC, dtype=np.int64)
        counts = np.zeros(TPC, dtype=np.int64)
        for nd in heavy_first:
            cand = np.flatnonzero(counts < CHUNK)
            t = cand[np.argmin(loads[cand])]
            tile_of_node[nd] = t
            slot_of_node[nd] = counts[t]
            counts[t] += 1
            loads[t] += indeg[nd]

    # Rebalance tiles so per-(core, tile, half) in-edge counts stay <= 8
    # chunks: a node's (nA, nB) split depends on its SOURCES' tile halves,
    # which the repacking itself changes -> iterate a few rounds.
    er0, ec0 = row[mask], col[mask]
    for rnd in range(6):
        repack = rnd < 3
        src_is_b = (tile_of_node[er0] >= HALF).astype(np.int64)
        na = np.bincount(ec0[src_is_b == 0], minlength=N).astype(np.int64)
        nb = np.bincount(ec0[src_is_b == 1], minlength=N).astype(np.int64)
        for c in range(NCORES):
            nodes_c = np.flatnonzero(node_core == c)
            A = np.zeros(TPC, dtype=np.int64)
            B = np.zeros(TPC, dtype=np.int64)
            cnt_t = np.zeros(TPC, dtype=np.int64)
            tile_members = {(c, t): [] for t in range(TPC)}
            if repack:
                order_c = nodes_c[
                    np.argsort(-(na[nodes_c] + nb[nodes_c]), kind="stable")
                ]
                for nd in order_c:
                    free = cnt_t < CHUNK
                    pa = A + na[nd]
                    pb = B + nb[nd]
                    score = (
                        np.maximum(pa, pb)
                        + 1_000_000 * (pa > 8 * CHUNK)
                        + 1_000_000 * (pb > 8 * CHUNK)
                    )
                    score[~free] = np.iinfo(np.int64).max
                    t = int(np.argmin(score))
                    tile_of_node[nd] = t
                    tile_members[(c, t)].append(nd)
                    cnt_t[t] += 1
                    A[t] += na[nd]
                    B[t] += nb[nd]
            else:
                for nd in nodes_c:
                    t = int(tile_of_node[nd])
                    tile_members[(c, t)].append(nd)
                    cnt_t[t] += 1
                    A[t] += na[nd]
                    B[t] += nb[nd]
            # repair: move nodes out of >8-chunk (tile, half) bins
            for _rep in range(400):
                worst = max(range(TPC), key=lambda t: max(A[t], B[t]))
                wv = max(A[worst], B[worst])
                if wv <= 8 * CHUNK:
                    break
                hv = 0 if A[worst] >= B[worst] else 1
                members = tile_members[(c, worst)]
                key = na if hv == 0 else nb
                members.sort(key=lambda nd: -key[nd])
                moved = False
                for nd in members[:24]:
                    va, vb = na[nd], nb[nd]
                    ok = (
                        (cnt_t < CHUNK)
                        & (A + va <= 8 * CHUNK)
                        & (B + vb <= 8 * CHUNK)
                    )
                    ok[worst] = False
                    cand = np.flatnonzero(ok)
                    if len(cand):
                        t2 = int(cand[np.argmin(np.maximum(A, B)[cand])])
                        members.remove(nd)
                        tile_members[(c, t2)].append(nd)
                        tile_of_node[nd] = t2
                        cnt_t[worst] -= 1
                        cnt_t[t2] += 1
                        A[worst] -= va
                        B[worst] -= vb
                        A[t2] += va
                        B[t2] += vb
                        moved = True
                        break
                if not moved:
                    break

    # final slot assignment from tile membership
    for c in range(NCORES):
        for t in range(TPC):
            members = np.flatnonzero((node_core == c) & (tile_of_node == t))
            slot_of_node[members] = np.arange(len(members))

    gslot = node_core * NROWS + tile_of_node * CHUNK + slot_of_node
    slot_node = np.full(NCORES * NROWS, -1, dtype=np.int64)
    slot_node[gslot] = np.arange(N)

    er, ec, ew = row[mask], col[mask], w[mask]
    dest_core = node_core[ec]
    dest_tile = tile_of_node[ec]
    dest_slot = slot_of_node[ec]
    src_gslot = gslot[er]
    src_core = src_gslot // NROWS
    src_loc = src_gslot % NROWS
    src_half = src_loc >= HROWS  # False=A (tiles 0-9), True=B
    # row index within the half-exchange buffer [NCORES*HROWS, fo]
    src_hrow = src_core * HROWS + np.where(src_half, src_loc - HROWS, src_loc)

    # global chunk counts per half
    cnt = np.zeros((2, NCORES, TPC), dtype=np.int64)
    np.add.at(cnt, (src_half.astype(np.int64), dest_core, dest_tile), 1)
    cpa = int(np.ceil(cnt[0].max() / CHUNK))
    cpb = int(np.ceil(cnt[1].max() / CHUNK))
    # per-(half, tile) REAL chunk count: max over cores, so one SPMD program
    # can gather/matmul only the needed chunks (pads within the last real
    # chunk point at row 0 with S weight 0).
    cpj = tuple(
        tuple(int(np.ceil(cnt[h, :, t].max() / CHUNK)) for t in range(TPC))
        for h in (0, 1)
    )

    cores = []
    for c in range(NCORES):
        m = dest_core == c
        halves = []
        for h, cp in ((0, cpa), (1, cpb)):
            mh = m & (src_half == (h == 1))
            t_arr = dest_tile[mh]
            s_arr = dest_slot[mh]
            src_arr = src_hrow[mh]
            w_arr = ew[mh]
            o = np.argsort(t_arr, kind="stable")
            t_arr, s_arr, src_arr, w_arr = t_arr[o], s_arr[o], src_arr[o], w_arr[o]
            epad = TPC * cp * CHUNK
            idx = np.zeros(epad, dtype=np.int16)
            sval = np.zeros(epad, dtype=np.float32)
            dloc = np.zeros(epad, dtype=np.int16)
            starts = np.searchsorted(t_arr, np.arange(TPC))
            ends = np.searchsorted(t_arr, np.arange(TPC) + 1)
            for t in range(TPC):
                a, b = starts[t], ends[t]
                base = t * cp * CHUNK
                idx[base : base + b - a] = src_arr[a:b]
                sval[base : base + b - a] = 2.0 * w_arr[a:b]
                dloc[base : base + b - a] = s_arr[a:b]
            halves.append((idx, sval, dloc))
        cores.append(halves)

    meta = dict(gslot=gslot, slot_node=slot_node, cpa=cpa, cpb=cpb, cpj=cpj)
    return meta, cores


def _pack_idx(idx, piece):
    """dma_gather index layout: flat gathered position i within a piece reads
    idx_sb[i % 16, i // 16] (column-major wrap over 16 partitions), pieces
    side by side along the free dim, replicated to 128 partitions."""
    epad = idx.shape[0]
    blocks = [
        np.ascontiguousarray(idx[g * piece : (g + 1) * piece].reshape(piece // 16, 16).T)
        for g in range(epad // piece)
    ]
    arr = np.concatenate(blocks, axis=1)
    return np.ascontiguousarray(np.tile(arr, (8, 1))).astype(np.int16)


def _build_sblocks(sval, dloc, epad):
    s = np.zeros((CHUNK, epad), dtype=np.float32)
    flat = np.arange(epad)
    j = flat // CHUNK
    kk = flat % CHUNK
    s[kk, j * CHUNK + dloc.astype(np.int64)] = sval
    return s.astype(BF16)


def _fuse_weights(inputs):
    """Per layer: (wf [kpad, f_out] = Wc[0]+Wl (+0 pad), wrest [4, kpad, f_out],
    bias [1, f_out] = bc+bl), all bf16, scaled by s_li/s_{li-1}."""
    out = []
    for li in range(3):
        kpad, f_out = _layer_dims(li)
        gain = SCALES[li] / (SCALES[li - 1] if li > 0 else 1.0)
        Wc = np.asarray(inputs[f"Wc{li}"], np.float32) * gain
        Wl = np.asarray(inputs[f"Wl{li}"], np.float32) * gain
        bc = np.asarray(inputs[f"bc{li}"], np.float32) * SCALES[li]
        bl = np.asarray(inputs[f"bl{li}"], np.float32) * SCALES[li]
        f_in, f_real = Wc.shape[1], Wc.shape[2]
        wf = np.zeros((kpad, f_out), np.float32)
        wf[:f_in, :f_real] = Wc[0] + Wl
        wrest = np.zeros((K - 1, kpad, f_out), np.float32)
        for k in range(1, K):
            wrest[k - 1, :f_in, :f_real] = Wc[k]
        bias = np.zeros((1, f_out), np.float32)
        bias[0, :f_real] = bc + bl
        out.append((wf.astype(BF16), wrest.astype(BF16), bias.astype(BF16)))
    return out


# ------------------------------------------------------------- device side ---


NQUEUES = 4


def build_bass(cpa, cpb, reps=1, ablate=(), cpj=None):
    if cpj is None:
        cpj = ((cpa,) * TPC, (cpb,) * TPC)
    import concourse.bacc as bacc
    import concourse.bass as bass
    import concourse.mybir as mybir
    import concourse.tile as tile
    from concourse import library_config

    dt = mybir.dt
    epad_a = TPC * cpa * CHUNK
    epad_b = TPC * cpb * CHUNK

    nc = bacc.Bacc(
        "TRN2",
        target_bir_lowering=False,
        debug=False,
        num_devices=NCORES,
        num_swdge_queues=NQUEUES,
    )

    # ---- I/O ----
    xT_d = nc.dram_tensor("xT", [KPAD_IN, NROWS], dt.bfloat16, kind="ExternalInput")
    idxA_d = nc.dram_tensor("idxA", [128, epad_a // 16], dt.int16, kind="ExternalInput")
    idxB_d = nc.dram_tensor("idxB", [128, epad_b // 16], dt.int16, kind="ExternalInput")
    sblkA_d = nc.dram_tensor("sblkA", [128, epad_a], dt.bfloat16, kind="ExternalInput")
    sblkB_d = nc.dram_tensor("sblkB", [128, epad_b], dt.bfloat16, kind="ExternalInput")
    ident_d = nc.dram_tensor("ident", [128, 128], dt.bfloat16, kind="ExternalInput")
    w_d = []
    for li in range(3):
        kpad, f_out = _layer_dims(li)
        w_d.append(
            (
                nc.dram_tensor(f"wf{li}", [kpad, f_out], dt.bfloat16, kind="ExternalInput"),
                nc.dram_tensor(
                    f"wr{li}", [K - 1, kpad, f_out], dt.bfloat16, kind="ExternalInput"
                ),
                nc.dram_tensor(f"bias{li}", [1, f_out], dt.bfloat16, kind="ExternalInput"),
            )
        )
    out_d = nc.dram_tensor("out_sh", [NROWS, F2REAL], dt.float32, kind="ExternalOutput")

    # internal DRAM for the split state exchange
    bnA8 = nc.dram_tensor("bnA8", [HROWS, HID], dt.float8e3, kind="Internal")
    bnB8 = nc.dram_tensor("bnB8", [HROWS, HID], dt.float8e3, kind="Internal")
    bfA8 = nc.dram_tensor(
        "bfA8", [NCORES * HROWS, HID], dt.float8e3, kind="Internal", addr_space="Shared"
    )
    bfB8 = nc.dram_tensor(
        "bfB8", [NCORES * HROWS, HID], dt.float8e3, kind="Internal", addr_space="Shared"
    )
    bnA2 = nc.dram_tensor("bnA2", [HROWS, F2REAL], dt.bfloat16, kind="Internal")
    bnB2 = nc.dram_tensor("bnB2", [HROWS, F2REAL], dt.bfloat16, kind="Internal")
    bfA264 = nc.dram_tensor(
        "bfA264", [NCORES * HROWS, F2REAL], dt.bfloat16, kind="Internal",
        addr_space="Shared",
    )
    bfB264 = nc.dram_tensor(
        "bfB264", [NCORES * HROWS, F2REAL], dt.bfloat16, kind="Internal",
        addr_space="Shared",
    )
    bfA2 = nc.dram_tensor("bfA2", [NCORES * HROWS, F2PAD], dt.bfloat16, kind="Internal")
    bfB2 = nc.dram_tensor("bfB2", [NCORES * HROWS, F2PAD], dt.bfloat16, kind="Internal")

    groups = [list(range(NCORES))]

    with tile.TileContext(nc) as tc:
        with (
            tc.tile_pool(name="const", bufs=1) as cpool,
            tc.tile_pool(name="hT", bufs=2) as hpool,
            tc.tile_pool(name="bstate", bufs=3) as bpool,
            tc.tile_pool(name="rpa", bufs=1) as rapool,
            tc.tile_pool(name="xbuf", bufs=2) as xpool,
            tc.tile_pool(name="x8buf", bufs=8) as x8pool,
            tc.tile_pool(name="small", bufs=2) as spool,
            tc.tile_pool(name="zpsum", bufs=2, space="PSUM") as zpool,
            tc.tile_pool(name="rpsum", bufs=3, space="PSUM") as rpool,
            tc.tile_pool(name="tpsum", bufs=2, space="PSUM") as tpool,
        ):
            nc.gpsimd.load_library(library_config.mlp)
            dummy_x = {}

            # ---- resident loads (hT0 + weights first: they gate layer-0 Z4) ----
            hT0 = cpool.tile([128, KPAD_IN // 128, NROWS], dt.bfloat16, tag="hT0")
            nc.sync.dma_start(hT0[:], xT_d.ap().rearrange("(c p) n -> p c n", p=128))

            w_sb = []
            for li in range(3):
                kpad, f_out = _layer_dims(li)
                kc = kpad // 128
                wf_sb = cpool.tile([128, kc, f_out], dt.bfloat16, tag=f"wf{li}")
                nc.sync.dma_start(
                    wf_sb[:], w_d[li][0].ap().rearrange("(c p) f -> p c f", p=128)
                )
                wr_sb = cpool.tile([128, K - 1, kc, f_out], dt.bfloat16, tag=f"wr{li}")
                nc.sync.dma_start(
                    wr_sb[:], w_d[li][1].ap().rearrange("k (c p) f -> p k c f", p=128)
                )
                bias_sb = cpool.tile([1, f_out], dt.bfloat16, tag=f"bias{li}")
                nc.sync.dma_start(bias_sb[:], w_d[li][2].ap())
                w_sb.append((wf_sb, wr_sb, bias_sb))

            sblkA_sb = cpool.tile([128, epad_a], dt.bfloat16, tag="sblkA")
            nc.sync.dma_start(sblkA_sb[:], sblkA_d.ap())
            sblkB_sb = cpool.tile([128, epad_b], dt.bfloat16, tag="sblkB")
            nc.sync.dma_start(sblkB_sb[:], sblkB_d.ap())
            idxA_sb = cpool.tile([128, epad_a // 16], dt.int16, tag="idxA")
            nc.sync.dma_start(idxA_sb[:], idxA_d.ap())
            idxB_sb = cpool.tile([128, epad_b // 16], dt.int16, tag="idxB")
            nc.sync.dma_start(idxB_sb[:], idxB_d.ap())
            ident_sb = cpool.tile([128, 128], dt.bfloat16, tag="ident")
            nc.sync.dma_start(ident_sb[:], ident_d.ap())
            ones_sb = cpool.tile([1, 128], dt.bfloat16, tag="ones")
            nc.vector.memset(ones_sb[:], 1.0)

            def dense_tile(zp, hT_in, li, widx, t, with_bias, stop=True):
                """psum[128 nodes, f_out] = h_tile @ W  (+ ones x bias)."""
                kpad, f_out = _layer_dims(li)
                kc = kpad // 128
                wf_sb, wr_sb, bias_sb = w_sb[li]
                zv = zp[:, :f_out]
                for c in range(kc):
                    lhsT = hT_in[:, c, t * 128 : (t + 1) * 128]
                    rhs = wf_sb[:, c, :] if widx == 0 else wr_sb[:, widx - 1, c, :]
                    nc.tensor.matmul(
                        zv,
                        lhsT,
                        rhs,
                        start=(c == 0),
                        stop=(stop and c == kc - 1 and not with_bias),
                    )
                if with_bias:
                    nc.tensor.matmul(
                        zv, ones_sb[:1, :], bias_sb[:1, :], start=False, stop=stop
                    )

            def ag_half(bounce, bfull, widen_to=None):
                if "coll" in ablate:
                    return
                with nc.named_scope("AG"):
                    nc.gpsimd.collective_compute(
                        "AllGather",
                        mybir.AluOpType.bypass,
                        replica_groups=groups,
                        ins=[bounce.ap().opt()],
                        outs=[bfull.ap().opt()],
                    )
                    if widen_to is not None:
                        nc.sync.dma_start(widen_to.ap()[:, :F2REAL], bfull.ap())

            def run_layer(li, hT_in, hT_out):
                kpad, f_out = _layer_dims(li)
                fo = f_out
                fp8 = li < 2
                fg = F2PAD if li == 2 else fo  # gather row width (256B granule)
                if fp8:
                    bnA, bnB, gfA, gfB = bnA8, bnB8, bfA8, bfB8
                    wdA = wdB = None
                    gsrcA, gsrcB = bfA8, bfB8
                else:
                    bnA, bnB, gfA, gfB = bnA2, bnB2, bfA264, bfB264
                    wdA, wdB = bfA2, bfB2
                    gsrcA, gsrcB = bfA2, bfB2

                def bounce_tile(t, src_tile):
                    bn = bnA if t < HALF else bnB
                    th = t % HALF
                    if fp8:
                        st = spool.tile([128, HID], dt.float8e3, tag="st8")
                        nc.vector.tensor_copy(st[:, :fo], src_tile)
                        nc.sync.dma_start(
                            bn.ap()[th * 128 : (th + 1) * 128, :fo], st[:, :fo]
                        )
                    else:
                        nc.sync.dma_start(
                            bn.ap()[th * 128 : (th + 1) * 128, :], src_tile
                        )

                def gather_group(g, which):
                    """One dma_gather for tile g's real chunks (cpj[which][g]).

                    X is always an fp8-typed [128, cp, 256] byte buffer
                    (256B per gathered row); layer 2's bf16 rows are bitcast
                    back at the matmul site."""
                    if which == 0:
                        cp, idx_sb, gsrc = cpa, idxA_sb, gsrcA
                    else:
                        cp, idx_sb, gsrc = cpb, idxB_sb, gsrcB
                    cpjv = cpj[which][g]
                    nidx = cpjv * CHUNK
                    if "gather" in ablate:
                        key = ("Xdummy", li, which)
                        if key not in dummy_x:
                            Xd = cpool.tile(
                                [128, cp, 256], dt.float8e3, tag=f"Xd{li}{which}"
                            )
                            nc.vector.memset(Xd[:], 0.0)
                            dummy_x[key] = Xd
                        return dummy_x[key]
                    X = x8pool.tile([128, cp, 256], dt.float8e3, tag="X8")
                    src_ap = gsrc.ap()
                    if not fp8:
                        src_ap = src_ap.bitcast(dt.float8e3)
                    col0 = g * cp * CHUNK // 16
                    nc.gpsimd.dma_gather(
                        X[:, :cpjv, :],
                        src_ap,
                        idx_sb[:, col0 : col0 + nidx // 16],
                        nidx,
                        nidx,
                        256,
                        single_packet=False,
                        queue_num=g % NQUEUES,
                    )
                    return X

                def xv(X, idx):
                    """Matmul rhs view of gathered row `idx`."""
                    if fp8:
                        return X[:, idx, :fo]
                    return X[:, idx, :].bitcast(dt.bfloat16)[:, :fo]

                def self_out(li, t, X, tl, rpa, b_prev2, hT_out):
                    """k==0: out = relu(Z0 + bias + 0.5*(2 L b1) - b2)."""
                    zp = zpool.tile([128, HID], dt.float32, tag="z")
                    dense_tile(zp, hT_in, li, 0, t, True)
                    z_sb = spool.tile([128, HID], dt.bfloat16, tag="zsb")
                    nc.vector.tensor_copy(z_sb[:, :fo], zp[:, :fo])
                    rp = rpool.tile([128, HID], dt.float32, tag="r")
                    nc.tensor.matmul(
                        rp[:, :fo],
                        ident_sb[:],
                        rpa[:, t, :fo],
                        start=True,
                        stop=("sparse" in ablate),
                    )
                    nj = cpj[1][t]
                    for j in range(nj):
                        if "sparse" in ablate:
                            break
                        e0 = (t * cpb + j) * CHUNK
                        nc.tensor.matmul(
                            rp[:, :fo],
                            sblkB_sb[:, e0 : e0 + CHUNK],
                            xv(X, tl * cpb + j),
                            start=False,
                            stop=(j == nj - 1),
                        )
                    # out = relu(Z0L + 0.5*P2 - b2)
                    a1 = spool.tile([128, HID], dt.float32, tag="a1")
                    nc.vector.tensor_scalar_mul(a1[:, :fo], rp[:, :fo], 0.5)
                    a2 = spool.tile([128, HID], dt.bfloat16, tag="ttmp")
                    nc.vector.tensor_sub(
                        a2[:, :fo], z_sb[:, :fo], b_prev2[:, t, :fo]
                    )
                    if li < 2:
                        h = spool.tile([128, HID], dt.bfloat16, tag="h")
                        nc.vector.tensor_add(h[:, :fo], a1[:, :fo], a2[:, :fo])
                        nc.vector.tensor_relu(h[:, :fo], h[:, :fo])
                        for c2 in range(fo // 128):
                            tp = tpool.tile([128, 128], dt.bfloat16, tag="tp")
                            nc.tensor.transpose(
                                tp[:],
                                h[:, c2 * 128 : (c2 + 1) * 128],
                                ident_sb[:],
                            )
                            nc.vector.tensor_copy(
                                hT_out[:, c2, t * 128 : (t + 1) * 128], tp[:]
                            )
                    else:
                        hf = spool.tile([128, F2REAL], dt.float32, tag="hf")
                        nc.vector.tensor_add(hf[:], a1[:, :fo], a2[:, :fo])
                        nc.vector.tensor_relu(hf[:], hf[:])
                        nc.sync.dma_start(
                            out_d.ap()[t * 128 : (t + 1) * 128, :], hf[:]
                        )

                # --- b4 = Z4, straight to bounce + SBUF state ---
                b4 = bpool.tile([128, TPC, HID], dt.bfloat16, tag="bst")
                with nc.named_scope(f"b4-L{li}"):
                    for t in range(TPC):
                        zp = zpool.tile([128, HID], dt.float32, tag="z")
                        dense_tile(zp, hT_in, li, 4, t, False)
                        nc.vector.tensor_copy(b4[:, t, :fo], zp[:, :fo])
                        bounce_tile(t, b4[:, t, :fo])
                        if t == HALF - 1:
                            ag_half(bnA, gfA, wdA)
                ag_half(bnB, gfB, wdB)

                b_prev2 = None  # b_{k+2}
                b_prev1 = b4  # b_{k+1} (already exchanged)
                for kth in (3, 2, 1, 0):
                    is_final = kth == 0
                    b_new = (
                        None
                        if is_final
                        else bpool.tile([128, TPC, HID], dt.bfloat16, tag="bst")
                    )
                    # ---- A phase: half-A sparse matmuls, spilled to SBUF ----
                    rpa = rapool.tile([128, TPC, HID], dt.bfloat16, tag="rpa")
                    with nc.named_scope(f"sparseA-L{li}k{kth}"):
                        for g in range(TPC // GTI):
                            if "sparse" in ablate:
                                break
                            X = gather_group(g, 0)
                            for tl in range(GTI):
                                t = g * GTI + tl
                                rp = rpool.tile([128, HID], dt.float32, tag="r")
                                nj = cpj[0][t]
                                for j in range(nj):
                                    e0 = (t * cpa + j) * CHUNK
                                    nc.tensor.matmul(
                                        rp[:, :fo],
                                        sblkA_sb[:, e0 : e0 + CHUNK],
                                        xv(X, tl * cpa + j),
                                        start=(j == 0),
                                        stop=(j == nj - 1),
                                    )
                                nc.vector.tensor_copy(rpa[:, t, :fo], rp[:, :fo])
                    # ---- B phase: dense Z + half-A inject + half-B sparse,
                    # all fused into one PSUM accumulation per tile ----
                    with nc.named_scope(f"sparseB-L{li}k{kth}"):
                        for g in range(TPC // GTI):
                            X = gather_group(g, 1)
                            for tl in range(GTI):
                                t = g * GTI + tl
                                if is_final:
                                    self_out(li, t, X, tl, rpa, b_prev2, hT_out)
                                    continue
                                rp = rpool.tile([128, HID], dt.float32, tag="r")
                                dense_tile(rp, hT_in, li, kth, t, False, stop=False)
                                nc.tensor.matmul(
                                    rp[:, :fo],
                                    ident_sb[:],
                                    rpa[:, t, :fo],
                                    start=False,
                                    stop=("sparse" in ablate),
                                )
                                nj = cpj[1][t]
                                for j in range(nj):
                                    if "sparse" in ablate:
                                        break
                                    e0 = (t * cpb + j) * CHUNK
                                    nc.tensor.matmul(
                                        rp[:, :fo],
                                        sblkB_sb[:, e0 : e0 + CHUNK],
                                        xv(X, tl * cpb + j),
                                        start=False,
                                        stop=(j == nj - 1),
                                    )
                                if kth == 3:
                                    nc.vector.tensor_copy(
                                        b_new[:, t, :fo], rp[:, :fo]
                                    )
                                else:
                                    nc.vector.tensor_sub(
                                        b_new[:, t, :fo],
                                        rp[:, :fo],
                                        b_prev2[:, t, :fo],
                                    )
                                bounce_tile(t, b_new[:, t, :fo])
                                if t == HALF - 1:
                                    ag_half(bnA, gfA, wdA)
                    if not is_final:
                        ag_half(bnB, gfB, wdB)
                        b_prev2 = b_prev1
                        b_prev1 = b_new

            for _ in range(reps):
                hT1 = hpool.tile([128, HID // 128, NROWS], dt.bfloat16, tag="hTn")
                run_layer(0, hT0, hT1)
                hT2 = hpool.tile([128, HID // 128, NROWS], dt.bfloat16, tag="hTn")
                run_layer(1, hT1, hT2)
                run_layer(2, hT2, None)

    nc.compile()
    return nc


# ----------------------------------------------------------------- runner ---

_CACHE = {}


def _get_nc(cpa, cpb, reps=1, ablate=(), cpj=None):
    key = (cpa, cpb, reps, tuple(ablate), cpj)
    if key not in _CACHE:
        _CACHE[key] = build_bass(cpa, cpb, reps, ablate=ablate, cpj=cpj)
    return _CACHE[key]


def make_in_maps(inputs):
    x = np.asarray(inputs["x"], np.float32)
    edge_index = np.asarray(inputs["edge_index"])
    meta, cores = _build_all(edge_index)
    gslot = meta["gslot"]
    cpa, cpb = meta["cpa"], meta["cpb"]
    epad_a = TPC * cpa * CHUNK
    epad_b = TPC * cpb * CHUNK

    weights = _fuse_weights(inputs)
    ident = np.eye(128, dtype=BF16)

    x_slot = np.zeros((NCORES * NROWS, KPAD_IN), np.float32)
    x_slot[gslot, :F_IN] = x

    in_maps = []
    for c in range(NCORES):
        (idxA, svalA, dlocA), (idxB, svalB, dlocB) = cores[c]
        m = {
            "xT": np.ascontiguousarray(
                x_slot[c * NROWS : (c + 1) * NROWS].T
            ).astype(BF16),
            "idxA": _pack_idx(idxA, cpa * CHUNK),
            "idxB": _pack_idx(idxB, cpb * CHUNK),
            "sblkA": _build_sblocks(svalA, dlocA, epad_a),
            "sblkB": _build_sblocks(svalB, dlocB, epad_b),
            "ident": ident,
        }
        for li in range(3):
            wf, wrest, bias = weights[li]
            m[f"wf{li}"] = wf
            m[f"wr{li}"] = wrest
            m[f"bias{li}"] = bias
        in_maps.append(m)
    return in_maps, meta


def assemble_output(results, meta):
    slot_node = meta["slot_node"]
    out_slot = np.concatenate([r["out_sh"] for r in results], axis=0)
    out = np.zeros((N, BOT), np.float32)
    valid = slot_node >= 0
    out[slot_node[valid]] = out_slot[valid][:, :BOT]
    return out


def kernel(**inputs):
    from concourse import bass_utils

    in_maps, meta = make_in_maps(inputs)
    nc = _get_nc(meta["cpa"], meta["cpb"], cpj=meta["cpj"])
    res = bass_utils.run_bass_kernel_spmd(nc, in_maps, core_ids=list(range(NCORES)))
    return assemble_output(res.results, meta)

